# revision 4
# baseline (speedup 1.0000x reference)
"""DenseCaptioner LSTM-gate kernel for 8 Trainium2 NeuronCores.

Role-split sharding (halves per-core HBM traffic vs. gate+batch-half
data parallelism):
  cores 0-3  run program VIS: visual + recurrent paths for gate g = core,
             full batch (two 128-row m-tiles)  -> partial logits [256,1024]
  cores 4-7  run program INP: input path for gate g = core-4, full batch
             -> partial logits [256,1024]
Host: logits[g] = vis_part[g] + inp_part[g] + b[g], then sigmoid/tanh gate
math and the prev_c recurrence.

Program structure (bf16 matmuls, fp32 PSUM accumulation):
  - activations are DMA'd per 128-wide k-tile so the first matmul starts
    ~1us in instead of waiting for the whole activation image
  - each "gated pair" streams two weight matrices into two PSUM pairs,
    the Hadamard product is formed by scalar-engine bounce copy + vector
    mul, and the transposed copy needed as the next level's lhsT is made
    with DMA-XBAR 128x128 transposes (PE and PSUM stay out of it)
  - vis phase order V-level1 -> U-level1 -> C-level2 -> level3 keeps the
    PE busy across junctions; U3+C3 share one PSUM accumulation so the
    final logits need a single PSUM->SBUF copy
"""

import numpy as np

import jax
from jax.experimental.shard_map import shard_map
from jax.sharding import Mesh, PartitionSpec

import concourse.mybir as mybir
import concourse.tile as tile
from concourse import bacc, bass2jax

B, X, V, MM, VH, H1, H2, G = 256, 12000, 4096, 1024, 1024, 1024, 1024, 4
XP = 12032  # X padded to a multiple of 128 (94 k-tiles)
N_CORES = 8
MT = 2      # m-tiles (batch 256 = 2 x 128)

DT_NAME = "bfloat16"  # matmul dtype: "float32r" or "bfloat16"

_cache = {}


def _mm_dt():
    return getattr(mybir.dt, DT_NAME)


def _np_dt():
    return mybir.dt.np(_mm_dt())


def build_program(role):
    """role "vis": visual+recurrent paths; "inp": input path. Full batch."""
    dt = _mm_dt()
    f32 = mybir.dt.float32
    n_chunk = 512  # max matmul free dim (one PSUM bank)

    nc = bacc.Bacc("TRN2", target_bir_lowering=False, debug=False)

    if role == "vis":
        act_specs = {"v1T": V, "v2T": V, "mT": MM, "hT": H2}
        w_specs = {"V1": V, "V2": V, "C1": VH, "C2": MM, "C3": H1,
                   "U1": H2, "U2": MM, "U3": H1}
    else:
        act_specs = {"xT": XP, "mT": MM}
        w_specs = {"W1": XP, "W2": MM, "W3": H1}

    acts_d = {
        name: nc.dram_tensor(name, [128, k // 128 * B], dt, kind="ExternalInput")
        for name, k in act_specs.items()
    }
    wt = {
        name: nc.dram_tensor(name, [k, H1], dt, kind="ExternalInput")
        for name, k in w_specs.items()
    }
    out = nc.dram_tensor("out", [B, H2], f32, kind="ExternalOutput")

    with tile.TileContext(nc) as tc:
        with (
            tc.tile_pool(name="acts", bufs=1) as acts,
            tc.tile_pool(name="wstream", bufs=8) as wstream,
            tc.tile_pool(name="inter", bufs=1) as inter,
            tc.tile_pool(name="ps", bufs=2, space="PSUM") as ps,
        ):
            # --- resident activations, loaded per k-tile ---
            def load_act(name):
                dram = acts_d[name]
                ktiles = act_specs[name] // 128
                t = acts.tile([128, ktiles * B], dt, tag=name)
                tv3 = t.rearrange("p (t x) -> p t x", x=MT * 128)
                dv3 = dram.ap().rearrange("p (t x) -> p t x", x=MT * 128)
                for k in range(ktiles):
                    nc.sync.dma_start(tv3[:, k], dv3[:, k])
                return t.rearrange("p (t m b) -> p t m b", m=MT, b=128)

            act_sb = {name: load_act(name) for name in act_specs}

            def stream_mm(psums, act, wname, start=True, stop=True):
                """psums[m][128, 1024] (+)= act_m.T @ W, streaming W k-tiles."""
                ktiles = w_specs[wname] // 128
                w_dram = wt[wname].ap().rearrange("(t p) n -> t p n", p=128)
                for k in range(ktiles):
                    w = wstream.tile([128, H1], dt, tag="w")
                    nc.sync.dma_start(w[:], w_dram[k])
                    for mi in range(MT):
                        for n in range(0, H1, n_chunk):
                            nc.tensor.matmul(
                                psums[mi][:, n:n + n_chunk],
                                act(k, mi),
                                w[:, n:n + n_chunk],
                                start=start and (k == 0),
                                stop=stop and (k == ktiles - 1),
                            )

            def hadamard_T(pa, pb):
                """qT[m] = transpose(pa[m] * pb[m]) as SBUF image
                [128, 8, 128] per m-tile via DMA-XBAR; frees pa/pb psum."""
                qTs = []
                for mi in range(MT):
                    bounce = inter.tile([128, H1], f32, tag="bounce", bufs=2)
                    nc.scalar.activation(
                        bounce[:], pb[mi][:], mybir.ActivationFunctionType.Copy
                    )
                    q = inter.tile([128, H1], dt, tag="q", bufs=2)
                    nc.vector.tensor_mul(q[:], pa[mi][:], bounce[:])
                    qT = inter.tile([128, (H1 // 128) * 128], dt, tag="qT", bufs=4)
                    qTv = qT.rearrange("p (t b) -> p t b", b=128)
                    for j in range(H1 // 128):
                        nc.sync.dma_start(
                            qTv[:, j, :], q[:, j * 128:(j + 1) * 128],
                            transpose=True,
                        )
                    qTs.append(qTv)
                return qTs

            def pair(a_name, w_a, b_name, w_b):
                pa = [ps.tile([128, H1], f32, tag="s1", name=f"pa{w_a}_{i}")
                      for i in range(MT)]
                stream_mm(pa, lambda k, mi: act_sb[a_name][:, k, mi, :], w_a)
                pb = [ps.tile([128, H1], f32, tag="s2", name=f"pb{w_b}_{i}")
                      for i in range(MT)]
                stream_mm(pb, lambda k, mi: act_sb[b_name][:, k, mi, :], w_b)
                return pa, pb

            def pairT(qT, w_a, b_name, w_b):
                pa = [ps.tile([128, H1], f32, tag="s1", name=f"pa{w_a}_{i}")
                      for i in range(MT)]
                stream_mm(pa, lambda k, mi: qT[mi][:, k, :], w_a)
                pb = [ps.tile([128, H1], f32, tag="s2", name=f"pb{w_b}_{i}")
                      for i in range(MT)]
                stream_mm(pb, lambda k, mi: act_sb[b_name][:, k, mi, :], w_b)
                return pa, pb

            # l3 lives in tag s2: the s2 pair slots free one bounce-copy
            # earlier than s1's (which wait for the vector mul)
            def l3_tiles():
                return [ps.tile([128, H2], f32, tag="s2", name=f"l3_{i}")
                        for i in range(MT)]

            if role == "vis":
                pv = pair("v1T", "V1", "v2T", "V2")
                qv = hadamard_T(*pv)        # junction hidden under U-level1
                pu = pair("hT", "U1", "mT", "U2")
                qu = hadamard_T(*pu)        # junction hidden under C-level2
                pc = pairT(qv, "C1", "mT", "C2")
                qc = hadamard_T(*pc)
                l3 = l3_tiles()
                # U3 first: quT is ready long before qcT
                stream_mm(l3, lambda k, mi: qu[mi][:, k, :], "U3",
                          start=True, stop=False)
                stream_mm(l3, lambda k, mi: qc[mi][:, k, :], "C3",
                          start=False, stop=True)
            else:
                px = pair("xT", "W1", "mT", "W2")
                qx = hadamard_T(*px)
                l3 = l3_tiles()
                stream_mm(l3, lambda k, mi: qx[mi][:, k, :], "W3")

            out_v = out.ap().rearrange("(m p) n -> m p n", p=128)
            for mi in range(MT):
                o = inter.tile([128, H2], f32, tag="osb", bufs=2)
                nc.scalar.activation(
                    o[:], l3[mi][:], mybir.ActivationFunctionType.Copy
                )
                nc.sync.dma_start(out_v[mi], o[:])

    nc.compile()
    return nc


def _make_runner(nc, devices):
    """Adapted from concourse.bass2jax.run_bass_via_pjrt: same lowering,
    but runs on an explicit device subset and returns unmaterialized jax
    arrays so two programs can be dispatched concurrently."""
    bass2jax.install_neuronx_cc_hook()

    assert nc.dbg_addr is None
    partition_name = (
        nc.partition_id_tensor.name if nc.partition_id_tensor else None
    )

    in_names, out_names, out_avals, zero_outs = [], [], [], []
    for alloc in nc.m.functions[0].allocations:
        if not isinstance(alloc, mybir.MemoryLocationSet):
            continue
        name = alloc.memorylocations[0].name
        if alloc.kind == "ExternalInput":
            if name != partition_name:
                in_names.append(name)
        elif alloc.kind == "ExternalOutput":
            shape = tuple(alloc.tensor_shape)
            dtype = mybir.dt.np(alloc.dtype)
            out_names.append(name)
            out_avals.append(jax.core.ShapedArray(shape, dtype))
            zero_outs.append(np.zeros(shape, dtype))
    n_params = len(in_names)
    n_outs = len(out_avals)
    in_names.extend(out_names)
    if partition_name is not None:
        in_names.append(partition_name)
    donate = tuple(range(n_params, n_params + n_outs))

    def _body(*args):
        operands = list(args)
        if partition_name is not None:
            operands.append(bass2jax.partition_id_tensor())
        outs = bass2jax._bass_exec_p.bind(
            *operands,
            out_avals=tuple(out_avals),
            in_names=tuple(in_names),
            out_names=tuple(out_names),
            lowering_input_output_aliases=(),
            sim_require_finite=True,
            sim_require_nnan=True,
            nc=nc,
        )
        return tuple(outs)

    n_cores = len(devices)
    mesh = Mesh(np.asarray(devices), ("core",))
    in_specs = (PartitionSpec("core"),) * (n_params + n_outs)
    out_specs = (PartitionSpec("core"),) * n_outs
    sharded = jax.jit(
        shard_map(
            _body, mesh=mesh, in_specs=in_specs, out_specs=out_specs,
            check_rep=False,
        ),
        donate_argnums=donate,
        keep_unused=True,
    )

    def run(in_maps):
        assert len(in_maps) == n_cores
        concat_in = [
            np.concatenate(
                [np.asarray(in_maps[c][name]) for c in range(n_cores)], axis=0
            )
            for name in in_names[:n_params]
        ]
        concat_zeros = [
            np.zeros((n_cores * z.shape[0], *z.shape[1:]), z.dtype)
            for z in zero_outs
        ]
        out_arrs = sharded(*concat_in, *concat_zeros)
        return out_names, out_avals, out_arrs

    return run


def _tile_actT(a, kdim):
    """[256 batch, K<=kdim] -> SBUF image [128, (kdim/128) * 256]:
    (p, (t*2+mi)*128+b) = a[mi*128+b, t*128+p], contiguous per partition."""
    ktiles = kdim // 128
    a = np.asarray(a, np.float32)
    if a.shape[1] < kdim:
        a = np.pad(a, ((0, 0), (0, kdim - a.shape[1])))
    # [2m, 128b, ktiles, 128p] -> [128p, ktiles, 2m, 128b]
    r = a.reshape(MT, 128, ktiles, 128).transpose(3, 2, 0, 1)
    return np.ascontiguousarray(r.reshape(128, ktiles * B), dtype=_np_dt())


def kernel(prev_h, prev_c, x, m, v1, v2, V1, V2, C1, C2, C3, W1, W2, W3, U1, U2, U3, b):
    npdt = _np_dt()
    if "runners" not in _cache:
        devs = jax.devices()
        nc_vis = build_program("vis")
        nc_inp = build_program("inp")
        _cache["runners"] = (
            _make_runner(nc_vis, devs[0:4]),
            _make_runner(nc_inp, devs[4:8]),
        )
        _cache["ncs"] = (nc_vis, nc_inp)
    run_vis, run_inp = _cache["runners"]

    v1T_img = _tile_actT(v1, V)
    v2T_img = _tile_actT(v2, V)
    mT_img = _tile_actT(m, MM)
    hT_img = _tile_actT(prev_h, H2)
    xT_img = _tile_actT(x, XP)

    vis_maps, inp_maps = [], []
    for g in range(G):
        vis_maps.append({
            "v1T": v1T_img, "v2T": v2T_img, "mT": mT_img, "hT": hT_img,
            "V1": np.ascontiguousarray(V1[g], dtype=npdt),
            "V2": np.ascontiguousarray(V2[g], dtype=npdt),
            "C1": np.ascontiguousarray(C1[g], dtype=npdt),
            "C2": np.ascontiguousarray(C2[g], dtype=npdt),
            "C3": np.ascontiguousarray(C3[g], dtype=npdt),
            "U1": np.ascontiguousarray(U1[g], dtype=npdt),
            "U2": np.ascontiguousarray(U2[g], dtype=npdt),
            "U3": np.ascontiguousarray(U3[g], dtype=npdt),
        })
        w1_pad = np.zeros((XP, H1), np.float32)
        w1_pad[:X] = np.asarray(W1[g], np.float32)
        inp_maps.append({
            "xT": xT_img, "mT": mT_img,
            "W1": np.ascontiguousarray(w1_pad, dtype=npdt),
            "W2": np.ascontiguousarray(W2[g], dtype=npdt),
            "W3": np.ascontiguousarray(W3[g], dtype=npdt),
        })

    _cache["last_in_maps"] = (vis_maps, inp_maps)

    # dispatch both programs; they run concurrently on disjoint cores
    vnames, vavals, vouts = run_vis(vis_maps)
    inames, iavals, iouts = run_inp(inp_maps)

    vis_out = np.asarray(vouts[0]).reshape(G, B, H2)
    inp_out = np.asarray(iouts[0]).reshape(G, B, H2)

    logits = vis_out + inp_out + np.asarray(b, np.float32)[:, None, :]

    def sigmoid(z):
        return 1.0 / (1.0 + np.exp(-z))

    i = sigmoid(logits[0])
    f = sigmoid(logits[1])
    o = sigmoid(logits[2])
    cg = np.tanh(logits[3])
    prev_c = np.asarray(prev_c, np.float32)
    new_c = f * prev_c + i * cg
    new_h = o * np.tanh(prev_c)
    return new_h.astype(np.float32), new_c.astype(np.float32)


# revision 6
# speedup vs baseline: 1.4220x; 1.4220x over previous
"""DenseCaptioner LSTM-gate kernel for 8 Trainium2 NeuronCores.

Role-split sharding (halves per-core HBM traffic vs. gate+batch-half
data parallelism):
  cores 0-3  run program VIS: visual + recurrent paths for gate g = core,
             full batch (two 128-row m-tiles)  -> partial logits [256,1024]
  cores 4-7  run program INP: input path for gate g = core-4, full batch
             -> partial logits [256,1024]
Host: logits[g] = vis_part[g] + inp_part[g] + b[g], then sigmoid/tanh gate
math and the prev_c recurrence.

Program structure (bf16 matmuls, fp32 PSUM accumulation):
  - DMA instruction dispatch costs ~0.6us serialized on the sync queue,
    so transfers are laddered [1,1,2,4,8,8...] k-tiles per chunk: small
    first chunks let the PE start ~2us in, big later chunks keep the
    dispatch count low (~60/program)
  - activation chunks are issued inline with their first-use weight
    stream (same k-tile ladder) so act availability tracks weight needs
  - each "gated pair" streams two weight matrices into the two PSUM
    pairs; the Hadamard is a scalar-engine bounce copy + vector mul; the
    transposed copy the next level needs as lhsT is ONE batched DMA-XBAR
    transpose per m-tile (m0 on the scalar DGE queue, m1 deferred on the
    sync queue past the next stream's dispatches to dodge FIFO blocking)
  - vis phase order V-level1 -> U-level1 -> C-level2 -> level3 hides
    junctions under independent matmul streams; U3+C3 share one PSUM
    accumulation so the logits need a single PSUM->SBUF copy at the end
"""

import numpy as np

import jax
from jax.experimental.shard_map import shard_map
from jax.sharding import Mesh, PartitionSpec

import concourse.mybir as mybir
import concourse.tile as tile
from concourse import bacc, bass2jax

B, X, V, MM, VH, H1, H2, G = 256, 12000, 4096, 1024, 1024, 1024, 1024, 4
XP = 12032  # X padded to a multiple of 128 (94 k-tiles)
N_CORES = 8
MT = 2      # m-tiles (batch 256 = 2 x 128)

DT_NAME = "bfloat16"  # matmul dtype: "float32r" or "bfloat16"

_cache = {}


def _mm_dt():
    return getattr(mybir.dt, DT_NAME)


def _np_dt():
    return mybir.dt.np(_mm_dt())


def _ladder(total):
    """Chunk sizes in k-tiles: small first for fast PE start, 8-wide after."""
    steps, k = [], 0
    for s in (1, 1, 2, 4):
        if k >= total:
            break
        s = min(s, total - k)
        steps.append(s)
        k += s
    while k < total:
        s = min(8, total - k)
        steps.append(s)
        k += s
    return steps


def build_program(role):
    """role "vis": visual+recurrent paths; "inp": input path. Full batch."""
    dt = _mm_dt()
    f32 = mybir.dt.float32
    n_chunk = 512  # max matmul free dim (one PSUM bank)

    nc = bacc.Bacc("TRN2", target_bir_lowering=False, debug=False)

    if role == "vis":
        act_specs = {"v1T": V, "v2T": V, "mT": MM, "hT": H2}
        w_specs = {"V1": V, "V2": V, "C1": VH, "C2": MM, "C3": H1,
                   "U1": H2, "U2": MM, "U3": H1}
    else:
        act_specs = {"xT": XP, "mT": MM}
        w_specs = {"W1": XP, "W2": MM, "W3": H1}

    acts_d = {
        name: nc.dram_tensor(name, [128, k // 128 * B], dt, kind="ExternalInput")
        for name, k in act_specs.items()
    }
    wt = {
        name: nc.dram_tensor(name, [k, H1], dt, kind="ExternalInput")
        for name, k in w_specs.items()
    }
    out = nc.dram_tensor("out", [B, H2], f32, kind="ExternalOutput")

    with tile.TileContext(nc) as tc:
        with (
            tc.tile_pool(name="acts", bufs=1) as acts,
            tc.tile_pool(name="wstream", bufs=2) as wstream,
            tc.tile_pool(name="inter", bufs=1) as inter,
            tc.tile_pool(name="ps", bufs=2, space="PSUM") as ps,
        ):
            def act_loader(name):
                """Resident act tile + per-chunk DMA issuer (cols of 256)."""
                ktiles = act_specs[name] // 128
                t = acts.tile([128, ktiles * B], dt, tag=name)
                dram = acts_d[name].ap()

                def load(k0, s):
                    nc.sync.dma_start(
                        t[:, k0 * B:(k0 + s) * B], dram[:, k0 * B:(k0 + s) * B]
                    )
                view4 = t.rearrange("p (t m b) -> p t m b", m=MT, b=128)
                return load, view4

            def stream_mm(psums, wname, act, act_load=None,
                          start=True, stop=True, m_outer=False):
                """psums[m][128, 1024] (+)= act_m.T @ W, laddered k chunks.
                m_outer: all m0 matmuls before m1 (weight chunks all live)."""
                total_kt = w_specs[wname] // 128
                w_ap = wt[wname].ap()
                steps = _ladder(total_kt)
                chunks, k0 = [], 0
                for s in steps:
                    if act_load is not None:
                        act_load(k0, s)
                    w = wstream.tile([128, s * H1], dt, tag=f"w{s}", bufs=2)
                    wv = w.rearrange("p (t n) -> p t n", n=H1)
                    nc.sync.dma_start(
                        wv[:], w_ap[k0 * 128:(k0 + s) * 128].rearrange(
                            "(t p) n -> p t n", p=128)
                    )
                    chunks.append((k0, s, wv))
                    k0 += s

                def emit(mi):
                    for (c0, s, wv) in chunks:
                        for t_ in range(s):
                            k = c0 + t_
                            for n in range(0, H1, n_chunk):
                                nc.tensor.matmul(
                                    psums[mi][:, n:n + n_chunk],
                                    act(k, mi),
                                    wv[:, t_, n:n + n_chunk],
                                    start=start and (k == 0),
                                    stop=stop and (k == total_kt - 1),
                                )
                if m_outer:
                    for mi in range(MT):
                        emit(mi)
                else:
                    for (c0, s, wv) in chunks:
                        for t_ in range(s):
                            k = c0 + t_
                            for mi in range(MT):
                                for n in range(0, H1, n_chunk):
                                    nc.tensor.matmul(
                                        psums[mi][:, n:n + n_chunk],
                                        act(k, mi),
                                        wv[:, t_, n:n + n_chunk],
                                        start=start and (k == 0),
                                        stop=stop and (k == total_kt - 1),
                                    )

            def hadamard_T(pa, pb):
                """qT[m] = transpose(pa[m] * pb[m]) as [128, 8, 128] SBUF
                image via one batched DMA-XBAR per m-tile. m0's transpose
                dispatches on the scalar DGE queue; m1's is returned as a
                closure to emit later on the sync queue (past the next
                stream's dispatches). Frees pa/pb psum."""
                qTs, lates = [], []
                for mi in range(MT):
                    bounce = inter.tile([128, H1], f32, tag="bounce", bufs=2)
                    nc.scalar.activation(
                        bounce[:], pb[mi][:], mybir.ActivationFunctionType.Copy
                    )
                    q = inter.tile([128, H1], dt, tag="q", bufs=2)
                    nc.vector.tensor_mul(q[:], pa[mi][:], bounce[:])
                    qT = inter.tile([128, (H1 // 128) * 128], dt, tag="qT", bufs=4)
                    qTv = qT.rearrange("p (t b) -> p t b", b=128)
                    if mi == 0:
                        nc.scalar.dma_start(qTv[:], q[:], transpose=True)
                    else:
                        lates.append(
                            lambda qTv=qTv, q=q:
                            nc.sync.dma_start(qTv[:], q[:], transpose=True)
                        )
                    qTs.append(qTv)

                def late():
                    for f in lates:
                        f()
                return qTs, late

            def psum_pair(tag, nm):
                return [ps.tile([128, H1], f32, tag=tag, name=f"{nm}_{i}")
                        for i in range(MT)]

            if role == "vis":
                ldv1, v1v = act_loader("v1T")
                ldv2, v2v = act_loader("v2T")
                ldm, mv = act_loader("mT")
                ldh, hv = act_loader("hT")

                pa = psum_pair("s1", "paV")
                stream_mm(pa, "V1", lambda k, mi: v1v[:, k, mi, :], ldv1)
                pb = psum_pair("s2", "pbV")
                stream_mm(pb, "V2", lambda k, mi: v2v[:, k, mi, :], ldv2)
                qv, late_v = hadamard_T(pa, pb)

                pu1 = psum_pair("s1", "pu1")
                stream_mm(pu1, "U1", lambda k, mi: hv[:, k, mi, :], ldh)
                late_v()
                pu2 = psum_pair("s2", "pu2")
                stream_mm(pu2, "U2", lambda k, mi: mv[:, k, mi, :], ldm)
                qu, late_u = hadamard_T(pu1, pu2)

                pc1 = psum_pair("s1", "pc1")
                stream_mm(pc1, "C1", lambda k, mi: qv[mi][:, k, :])
                late_u()
                pc2 = psum_pair("s2", "pc2")
                stream_mm(pc2, "C2", lambda k, mi: mv[:, k, mi, :])
                qc, late_c = hadamard_T(pc1, pc2)

                l3 = psum_pair("s2", "l3")
                stream_mm(l3, "U3", lambda k, mi: qu[mi][:, k, :],
                          start=True, stop=False)
                late_c()
                stream_mm(l3, "C3", lambda k, mi: qc[mi][:, k, :],
                          start=False, stop=True)
            else:
                ldx, xv = act_loader("xT")
                ldm, mv = act_loader("mT")

                pa = psum_pair("s1", "px")
                stream_mm(pa, "W1", lambda k, mi: xv[:, k, mi, :], ldx)
                pb = psum_pair("s2", "pm")
                stream_mm(pb, "W2", lambda k, mi: mv[:, k, mi, :], ldm)
                qx, late_x = hadamard_T(pa, pb)

                l3 = psum_pair("s2", "l3")
                # m_outer: m0's W3 pass runs while m1's transpose finishes
                late_x()
                stream_mm(l3, "W3", lambda k, mi: qx[mi][:, k, :],
                          m_outer=True)

            out_v = out.ap().rearrange("(m p) n -> m p n", p=128)
            for mi in range(MT):
                o = inter.tile([128, H2], f32, tag="osb", bufs=2)
                nc.vector.tensor_copy(o[:], l3[mi][:])
                nc.sync.dma_start(out_v[mi], o[:])

    nc.compile()
    return nc


def _make_runner(nc, devices):
    """Adapted from concourse.bass2jax.run_bass_via_pjrt: same lowering,
    but runs on an explicit device subset and returns unmaterialized jax
    arrays so two programs can be dispatched concurrently."""
    bass2jax.install_neuronx_cc_hook()

    assert nc.dbg_addr is None
    partition_name = (
        nc.partition_id_tensor.name if nc.partition_id_tensor else None
    )

    in_names, out_names, out_avals, zero_outs = [], [], [], []
    for alloc in nc.m.functions[0].allocations:
        if not isinstance(alloc, mybir.MemoryLocationSet):
            continue
        name = alloc.memorylocations[0].name
        if alloc.kind == "ExternalInput":
            if name != partition_name:
                in_names.append(name)
        elif alloc.kind == "ExternalOutput":
            shape = tuple(alloc.tensor_shape)
            dtype = mybir.dt.np(alloc.dtype)
            out_names.append(name)
            out_avals.append(jax.core.ShapedArray(shape, dtype))
            zero_outs.append(np.zeros(shape, dtype))
    n_params = len(in_names)
    n_outs = len(out_avals)
    in_names.extend(out_names)
    if partition_name is not None:
        in_names.append(partition_name)
    donate = tuple(range(n_params, n_params + n_outs))

    def _body(*args):
        operands = list(args)
        if partition_name is not None:
            operands.append(bass2jax.partition_id_tensor())
        outs = bass2jax._bass_exec_p.bind(
            *operands,
            out_avals=tuple(out_avals),
            in_names=tuple(in_names),
            out_names=tuple(out_names),
            lowering_input_output_aliases=(),
            sim_require_finite=True,
            sim_require_nnan=True,
            nc=nc,
        )
        return tuple(outs)

    n_cores = len(devices)
    mesh = Mesh(np.asarray(devices), ("core",))
    in_specs = (PartitionSpec("core"),) * (n_params + n_outs)
    out_specs = (PartitionSpec("core"),) * n_outs
    sharded = jax.jit(
        shard_map(
            _body, mesh=mesh, in_specs=in_specs, out_specs=out_specs,
            check_rep=False,
        ),
        donate_argnums=donate,
        keep_unused=True,
    )

    def run(in_maps):
        assert len(in_maps) == n_cores
        concat_in = [
            np.concatenate(
                [np.asarray(in_maps[c][name]) for c in range(n_cores)], axis=0
            )
            for name in in_names[:n_params]
        ]
        concat_zeros = [
            np.zeros((n_cores * z.shape[0], *z.shape[1:]), z.dtype)
            for z in zero_outs
        ]
        out_arrs = sharded(*concat_in, *concat_zeros)
        return out_names, out_avals, out_arrs

    return run


def _tile_actT(a, kdim):
    """[256 batch, K<=kdim] -> SBUF image [128, (kdim/128) * 256]:
    (p, (t*2+mi)*128+b) = a[mi*128+b, t*128+p], contiguous per partition."""
    ktiles = kdim // 128
    a = np.asarray(a, np.float32)
    if a.shape[1] < kdim:
        a = np.pad(a, ((0, 0), (0, kdim - a.shape[1])))
    # [2m, 128b, ktiles, 128p] -> [128p, ktiles, 2m, 128b]
    r = a.reshape(MT, 128, ktiles, 128).transpose(3, 2, 0, 1)
    return np.ascontiguousarray(r.reshape(128, ktiles * B), dtype=_np_dt())


def kernel(prev_h, prev_c, x, m, v1, v2, V1, V2, C1, C2, C3, W1, W2, W3, U1, U2, U3, b):
    npdt = _np_dt()
    if "runners" not in _cache:
        devs = jax.devices()
        nc_vis = build_program("vis")
        nc_inp = build_program("inp")
        _cache["runners"] = (
            _make_runner(nc_vis, devs[0:4]),
            _make_runner(nc_inp, devs[4:8]),
        )
        _cache["ncs"] = (nc_vis, nc_inp)
    run_vis, run_inp = _cache["runners"]

    v1T_img = _tile_actT(v1, V)
    v2T_img = _tile_actT(v2, V)
    mT_img = _tile_actT(m, MM)
    hT_img = _tile_actT(prev_h, H2)
    xT_img = _tile_actT(x, XP)

    vis_maps, inp_maps = [], []
    for g in range(G):
        vis_maps.append({
            "v1T": v1T_img, "v2T": v2T_img, "mT": mT_img, "hT": hT_img,
            "V1": np.ascontiguousarray(V1[g], dtype=npdt),
            "V2": np.ascontiguousarray(V2[g], dtype=npdt),
            "C1": np.ascontiguousarray(C1[g], dtype=npdt),
            "C2": np.ascontiguousarray(C2[g], dtype=npdt),
            "C3": np.ascontiguousarray(C3[g], dtype=npdt),
            "U1": np.ascontiguousarray(U1[g], dtype=npdt),
            "U2": np.ascontiguousarray(U2[g], dtype=npdt),
            "U3": np.ascontiguousarray(U3[g], dtype=npdt),
        })
        w1_pad = np.zeros((XP, H1), np.float32)
        w1_pad[:X] = np.asarray(W1[g], np.float32)
        inp_maps.append({
            "xT": xT_img, "mT": mT_img,
            "W1": np.ascontiguousarray(w1_pad, dtype=npdt),
            "W2": np.ascontiguousarray(W2[g], dtype=npdt),
            "W3": np.ascontiguousarray(W3[g], dtype=npdt),
        })

    _cache["last_in_maps"] = (vis_maps, inp_maps)

    # dispatch both programs; they run concurrently on disjoint cores
    vnames, vavals, vouts = run_vis(vis_maps)
    inames, iavals, iouts = run_inp(inp_maps)

    vis_out = np.asarray(vouts[0]).reshape(G, B, H2)
    inp_out = np.asarray(iouts[0]).reshape(G, B, H2)

    logits = vis_out + inp_out + np.asarray(b, np.float32)[:, None, :]

    def sigmoid(z):
        return 1.0 / (1.0 + np.exp(-z))

    i = sigmoid(logits[0])
    f = sigmoid(logits[1])
    o = sigmoid(logits[2])
    cg = np.tanh(logits[3])
    prev_c = np.asarray(prev_c, np.float32)
    new_c = f * prev_c + i * cg
    new_h = o * np.tanh(prev_c)
    return new_h.astype(np.float32), new_c.astype(np.float32)


# revision 10
# speedup vs baseline: 1.4578x; 1.0252x over previous
"""DenseCaptioner LSTM-gate kernel for 8 Trainium2 NeuronCores.

Role-split sharding (halves per-core HBM traffic vs. gate+batch-half
data parallelism):
  cores 0-3  run program VIS: visual + recurrent paths for gate g = core,
             full batch (two 128-row m-tiles)  -> partial logits [256,1024]
  cores 4-7  run program INP: input path for gate g = core-4, full batch
             -> partial logits [256,1024]
Host: logits[g] = vis_part[g] + inp_part[g] + b[g], then sigmoid/tanh gate
math and the prev_c recurrence.

Program structure (bf16 matmuls, fp32 PSUM accumulation):
  - DMA instruction dispatch costs ~0.6us serialized on the sync queue,
    so transfers are laddered [1,1,2,4,8,8...] k-tiles per chunk: small
    first chunks let the PE start ~2us in, big later chunks keep the
    dispatch count low (~60/program)
  - activation chunks are issued inline with their first-use weight
    stream (same k-tile ladder) so act availability tracks weight needs
  - each "gated pair" streams two weight matrices into the two PSUM
    pairs; the Hadamard is a scalar-engine bounce copy + vector mul; the
    transposed copy the next level needs as lhsT is ONE batched DMA-XBAR
    transpose per m-tile (m0 on the scalar DGE queue, m1 deferred on the
    sync queue past the next stream's dispatches to dodge FIFO blocking)
  - vis phase order V-level1 -> U-level1 -> C-level2 -> level3 hides
    junctions under independent matmul streams; U3+C3 share one PSUM
    accumulation so the logits need a single PSUM->SBUF copy at the end
"""

import numpy as np

import jax
from jax.experimental.shard_map import shard_map
from jax.sharding import Mesh, PartitionSpec

import concourse.mybir as mybir
import concourse.tile as tile
from concourse import bacc, bass2jax

B, X, V, MM, VH, H1, H2, G = 256, 12000, 4096, 1024, 1024, 1024, 1024, 4
XP = 12032  # X padded to a multiple of 128 (94 k-tiles)
N_CORES = 8
MT = 2      # m-tiles (batch 256 = 2 x 128)

DT_NAME = "bfloat16"  # matmul dtype: "float32r" or "bfloat16"

_cache = {}


def _mm_dt():
    return getattr(mybir.dt, DT_NAME)


def _np_dt():
    return mybir.dt.np(_mm_dt())


def _ladder(total):
    """Chunk sizes in k-tiles: small first for fast PE start, 4-wide after
    (constant size so the DMA stream never falls behind a growth step)."""
    steps, k = [], 0
    for s in (1, 1, 2):
        if k >= total:
            break
        s = min(s, total - k)
        steps.append(s)
        k += s
    while k < total:
        s = min(4, total - k)
        steps.append(s)
        k += s
    return steps


def build_program(role):
    """role "vis": visual+recurrent paths; "inp": input path. Full batch."""
    dt = _mm_dt()
    f32 = mybir.dt.float32
    n_chunk = 512  # max matmul free dim (one PSUM bank)

    nc = bacc.Bacc("TRN2", target_bir_lowering=False, debug=False)

    if role == "vis":
        act_specs = {"v1T": V, "v2T": V, "mT": MM, "hT": H2}
        w_specs = {"V1": V, "V2": V, "C1": VH, "C2": MM, "C3": H1,
                   "U1": H2, "U2": MM, "U3": H1}
    else:
        act_specs = {"xT": XP, "mT": MM}
        w_specs = {"W1": XP, "W2": MM, "W3": H1}

    acts_d = {
        name: nc.dram_tensor(name, [128, k // 128 * B], dt, kind="ExternalInput")
        for name, k in act_specs.items()
    }
    wt = {
        name: nc.dram_tensor(name, [k, H1], dt, kind="ExternalInput")
        for name, k in w_specs.items()
    }
    out = nc.dram_tensor("out", [B, H2], f32, kind="ExternalOutput")

    with tile.TileContext(nc) as tc:
        with (
            tc.tile_pool(name="acts", bufs=1) as acts,
            tc.tile_pool(name="wstream", bufs=2) as wstream,
            tc.tile_pool(name="inter", bufs=1) as inter,
            tc.tile_pool(name="ps", bufs=2, space="PSUM") as ps,
        ):
            def act_loader(name):
                """Resident act tile + per-chunk DMA issuer (cols of 256)."""
                ktiles = act_specs[name] // 128
                t = acts.tile([128, ktiles * B], dt, tag=name)
                dram = acts_d[name].ap()

                def load(k0, s):
                    nc.sync.dma_start(
                        t[:, k0 * B:(k0 + s) * B], dram[:, k0 * B:(k0 + s) * B]
                    )
                view4 = t.rearrange("p (t m b) -> p t m b", m=MT, b=128)
                return load, view4

            def stream_mm(psums, wname, act, act_load=None,
                          start=True, stop=True, m_outer=False):
                """psums[m][128, 1024] (+)= act_m.T @ W, laddered k chunks.
                m_outer: all m0 matmuls before m1 (weight chunks all live)."""
                total_kt = w_specs[wname] // 128
                w_ap = wt[wname].ap()
                steps = _ladder(total_kt)
                chunks, k0 = [], 0
                for s in steps:
                    if act_load is not None:
                        act_load(k0, s)
                    w = wstream.tile([128, s * H1], dt, tag=f"w{s}",
                                     bufs=3 if s == 4 else 2)
                    wv = w.rearrange("p (t n) -> p t n", n=H1)
                    nc.sync.dma_start(
                        wv[:], w_ap[k0 * 128:(k0 + s) * 128].rearrange(
                            "(t p) n -> p t n", p=128)
                    )
                    chunks.append((k0, s, wv))
                    k0 += s

                def emit(mi):
                    for (c0, s, wv) in chunks:
                        for t_ in range(s):
                            k = c0 + t_
                            for n in range(0, H1, n_chunk):
                                nc.tensor.matmul(
                                    psums[mi][:, n:n + n_chunk],
                                    act(k, mi),
                                    wv[:, t_, n:n + n_chunk],
                                    start=start and (k == 0),
                                    stop=stop and (k == total_kt - 1),
                                )
                if m_outer:
                    for mi in range(MT):
                        emit(mi)
                else:
                    for (c0, s, wv) in chunks:
                        for t_ in range(s):
                            k = c0 + t_
                            for mi in range(MT):
                                for n in range(0, H1, n_chunk):
                                    nc.tensor.matmul(
                                        psums[mi][:, n:n + n_chunk],
                                        act(k, mi),
                                        wv[:, t_, n:n + n_chunk],
                                        start=start and (k == 0),
                                        stop=stop and (k == total_kt - 1),
                                    )

            def hadamard_T(pa, pb):
                """qT[m] = transpose(pa[m] * pb[m]) as [128, 8, 128] SBUF
                image via one batched DMA-XBAR per m-tile. m0's transpose
                dispatches on the scalar DGE queue; m1's is returned as a
                closure to emit later on the sync queue (past the next
                stream's dispatches). Frees pa/pb psum."""
                qTs, lates = [], []
                for mi in range(MT):
                    bounce = inter.tile([128, H1], f32, tag="bounce", bufs=2)
                    nc.scalar.activation(
                        bounce[:], pb[mi][:], mybir.ActivationFunctionType.Copy
                    )
                    q = inter.tile([128, H1], dt, tag="q", bufs=2)
                    nc.vector.tensor_mul(q[:], pa[mi][:], bounce[:])
                    qT = inter.tile([128, (H1 // 128) * 128], dt, tag="qT", bufs=4)
                    qTv = qT.rearrange("p (t b) -> p t b", b=128)
                    lates.append(
                        lambda qTv=qTv, q=q:
                        nc.sync.dma_start(qTv[:], q[:], transpose=True)
                    )
                    qTs.append(qTv)

                def late():
                    for f in lates:
                        f()
                return qTs, late

            def psum_pair(tag, nm):
                return [ps.tile([128, H1], f32, tag=tag, name=f"{nm}_{i}")
                        for i in range(MT)]

            if role == "vis":
                ldv1, v1v = act_loader("v1T")
                ldv2, v2v = act_loader("v2T")
                ldm, mv = act_loader("mT")
                ldh, hv = act_loader("hT")

                pa = psum_pair("s1", "paV")
                stream_mm(pa, "V1", lambda k, mi: v1v[:, k, mi, :], ldv1)
                pb = psum_pair("s2", "pbV")
                stream_mm(pb, "V2", lambda k, mi: v2v[:, k, mi, :], ldv2)
                qv, late_v = hadamard_T(pa, pb)

                pu1 = psum_pair("s1", "pu1")
                stream_mm(pu1, "U1", lambda k, mi: hv[:, k, mi, :], ldh)
                pu2 = psum_pair("s2", "pu2")
                stream_mm(pu2, "U2", lambda k, mi: mv[:, k, mi, :], ldm)
                late_v()
                qu, late_u = hadamard_T(pu1, pu2)

                pc1 = psum_pair("s1", "pc1")
                stream_mm(pc1, "C1", lambda k, mi: qv[mi][:, k, :])
                late_u()
                pc2 = psum_pair("s2", "pc2")
                stream_mm(pc2, "C2", lambda k, mi: mv[:, k, mi, :])
                qc, late_c = hadamard_T(pc1, pc2)

                l3 = psum_pair("s2", "l3")
                stream_mm(l3, "U3", lambda k, mi: qu[mi][:, k, :],
                          start=True, stop=False)
                late_c()
                stream_mm(l3, "C3", lambda k, mi: qc[mi][:, k, :],
                          start=False, stop=True)
            else:
                ldx, xv = act_loader("xT")
                ldm, mv = act_loader("mT")

                pa = psum_pair("s1", "px")
                stream_mm(pa, "W1", lambda k, mi: xv[:, k, mi, :], ldx)
                pb = psum_pair("s2", "pm")
                stream_mm(pb, "W2", lambda k, mi: mv[:, k, mi, :], ldm)
                qx, late_x = hadamard_T(pa, pb)

                l3 = psum_pair("s2", "l3")
                # m_outer: m0's W3 pass runs while m1's transpose finishes
                late_x()
                stream_mm(l3, "W3", lambda k, mi: qx[mi][:, k, :],
                          m_outer=True)

            out_v = out.ap().rearrange("(m p) n -> m p n", p=128)
            for mi in range(MT):
                o = inter.tile([128, H2], f32, tag="osb", bufs=2)
                nc.vector.tensor_copy(o[:], l3[mi][:])
                nc.sync.dma_start(out_v[mi], o[:])

    nc.compile()
    return nc


def _make_runner(nc, devices):
    """Adapted from concourse.bass2jax.run_bass_via_pjrt: same lowering,
    but runs on an explicit device subset and returns unmaterialized jax
    arrays so two programs can be dispatched concurrently."""
    bass2jax.install_neuronx_cc_hook()

    assert nc.dbg_addr is None
    partition_name = (
        nc.partition_id_tensor.name if nc.partition_id_tensor else None
    )

    in_names, out_names, out_avals, zero_outs = [], [], [], []
    for alloc in nc.m.functions[0].allocations:
        if not isinstance(alloc, mybir.MemoryLocationSet):
            continue
        name = alloc.memorylocations[0].name
        if alloc.kind == "ExternalInput":
            if name != partition_name:
                in_names.append(name)
        elif alloc.kind == "ExternalOutput":
            shape = tuple(alloc.tensor_shape)
            dtype = mybir.dt.np(alloc.dtype)
            out_names.append(name)
            out_avals.append(jax.core.ShapedArray(shape, dtype))
            zero_outs.append(np.zeros(shape, dtype))
    n_params = len(in_names)
    n_outs = len(out_avals)
    in_names.extend(out_names)
    if partition_name is not None:
        in_names.append(partition_name)
    donate = tuple(range(n_params, n_params + n_outs))

    def _body(*args):
        operands = list(args)
        if partition_name is not None:
            operands.append(bass2jax.partition_id_tensor())
        outs = bass2jax._bass_exec_p.bind(
            *operands,
            out_avals=tuple(out_avals),
            in_names=tuple(in_names),
            out_names=tuple(out_names),
            lowering_input_output_aliases=(),
            sim_require_finite=True,
            sim_require_nnan=True,
            nc=nc,
        )
        return tuple(outs)

    n_cores = len(devices)
    mesh = Mesh(np.asarray(devices), ("core",))
    in_specs = (PartitionSpec("core"),) * (n_params + n_outs)
    out_specs = (PartitionSpec("core"),) * n_outs
    sharded = jax.jit(
        shard_map(
            _body, mesh=mesh, in_specs=in_specs, out_specs=out_specs,
            check_rep=False,
        ),
        donate_argnums=donate,
        keep_unused=True,
    )

    def run(in_maps):
        assert len(in_maps) == n_cores
        concat_in = [
            np.concatenate(
                [np.asarray(in_maps[c][name]) for c in range(n_cores)], axis=0
            )
            for name in in_names[:n_params]
        ]
        concat_zeros = [
            np.zeros((n_cores * z.shape[0], *z.shape[1:]), z.dtype)
            for z in zero_outs
        ]
        out_arrs = sharded(*concat_in, *concat_zeros)
        return out_names, out_avals, out_arrs

    return run


def _tile_actT(a, kdim):
    """[256 batch, K<=kdim] -> SBUF image [128, (kdim/128) * 256]:
    (p, (t*2+mi)*128+b) = a[mi*128+b, t*128+p], contiguous per partition."""
    ktiles = kdim // 128
    a = np.asarray(a, np.float32)
    if a.shape[1] < kdim:
        a = np.pad(a, ((0, 0), (0, kdim - a.shape[1])))
    # [2m, 128b, ktiles, 128p] -> [128p, ktiles, 2m, 128b]
    r = a.reshape(MT, 128, ktiles, 128).transpose(3, 2, 0, 1)
    return np.ascontiguousarray(r.reshape(128, ktiles * B), dtype=_np_dt())


def kernel(prev_h, prev_c, x, m, v1, v2, V1, V2, C1, C2, C3, W1, W2, W3, U1, U2, U3, b):
    npdt = _np_dt()
    if "runners" not in _cache:
        devs = jax.devices()
        nc_vis = build_program("vis")
        nc_inp = build_program("inp")
        _cache["runners"] = (
            _make_runner(nc_vis, devs[0:4]),
            _make_runner(nc_inp, devs[4:8]),
        )
        _cache["ncs"] = (nc_vis, nc_inp)
    run_vis, run_inp = _cache["runners"]

    v1T_img = _tile_actT(v1, V)
    v2T_img = _tile_actT(v2, V)
    mT_img = _tile_actT(m, MM)
    hT_img = _tile_actT(prev_h, H2)
    xT_img = _tile_actT(x, XP)

    vis_maps, inp_maps = [], []
    for g in range(G):
        vis_maps.append({
            "v1T": v1T_img, "v2T": v2T_img, "mT": mT_img, "hT": hT_img,
            "V1": np.ascontiguousarray(V1[g], dtype=npdt),
            "V2": np.ascontiguousarray(V2[g], dtype=npdt),
            "C1": np.ascontiguousarray(C1[g], dtype=npdt),
            "C2": np.ascontiguousarray(C2[g], dtype=npdt),
            "C3": np.ascontiguousarray(C3[g], dtype=npdt),
            "U1": np.ascontiguousarray(U1[g], dtype=npdt),
            "U2": np.ascontiguousarray(U2[g], dtype=npdt),
            "U3": np.ascontiguousarray(U3[g], dtype=npdt),
        })
        w1_pad = np.zeros((XP, H1), np.float32)
        w1_pad[:X] = np.asarray(W1[g], np.float32)
        inp_maps.append({
            "xT": xT_img, "mT": mT_img,
            "W1": np.ascontiguousarray(w1_pad, dtype=npdt),
            "W2": np.ascontiguousarray(W2[g], dtype=npdt),
            "W3": np.ascontiguousarray(W3[g], dtype=npdt),
        })

    _cache["last_in_maps"] = (vis_maps, inp_maps)

    # dispatch both programs; they run concurrently on disjoint cores
    vnames, vavals, vouts = run_vis(vis_maps)
    inames, iavals, iouts = run_inp(inp_maps)

    vis_out = np.asarray(vouts[0]).reshape(G, B, H2)
    inp_out = np.asarray(iouts[0]).reshape(G, B, H2)

    logits = vis_out + inp_out + np.asarray(b, np.float32)[:, None, :]

    def sigmoid(z):
        return 1.0 / (1.0 + np.exp(-z))

    i = sigmoid(logits[0])
    f = sigmoid(logits[1])
    o = sigmoid(logits[2])
    cg = np.tanh(logits[3])
    prev_c = np.asarray(prev_c, np.float32)
    new_c = f * prev_c + i * cg
    new_h = o * np.tanh(prev_c)
    return new_h.astype(np.float32), new_c.astype(np.float32)
